# revision 12
# baseline (speedup 1.0000x reference)
"""Trainium2 Bass kernel for nn_GuidedSampler (vq_codebook).

Math: reference computes, per batch row b
    kv[k]   = W_k @ f_b                (K=64 candidate 1x1-conv outputs)
    d2[b,k] = ||kv[k] - q_b||^2
    code_b  = argmin_k d2[b,k]
    sel_b   = kv[code_b]
    loss    = mean((sel - q)^2) = sum_b d2[b, code_b] / (B*DQ*H*W)

Instead of materializing kv (512 MB), use the Gram identity
    d2[b,k] = <M_k, C_b> - 2<W_k, G_b> + ||q_b||^2
with  M_k = W_k^T W_k   (host-precomputed, k-indexed 64x64)
      [C_b G_b; G_b^T qq_b] = g_b g_b^T,  g_b = [f_b; q_b]  (72 x HW)
computed on-device as 32 accumulating 128-contraction matmuls over a
host-pre-transposed gT layout.  Then 80 small accumulating matmuls with a
host-built constant A contract the stats against (M | -2W | I_qq) to give the
full distance matrix [8 rows, 64 codes] per core, argmin via max_with_indices,
and a final gather-free selection pass sel_b = (sum_k onehot W_k^T)^T f_b.

Sharding: data-parallel over batch, 8 rows per core, weights replicated.
"""

import numpy as np

import concourse.bass as bass
import concourse.bacc as bacc
from concourse import mybir
from concourse.tile import TileContext
from concourse.bass_utils import run_bass_kernel_spmd

B, DIM, DQ, H, W, K = 64, 64, 8, 64, 64, 64
HW = H * W                # 4096
NCORES = 8
BL = B // NCORES          # 8 batch rows per core
GC = DIM + DQ             # 72 stats dim
NCHUNK = HW // 128        # 32 contraction chunks for stats
NJ = DIM + DQ + DQ        # 80 distance-accumulation steps
NFC = HW // 512           # 8 free-dim chunks for the selection matmul

F32 = mybir.dt.float32
U32 = mybir.dt.uint32

TRACE = False             # set by test.py for profiling runs
LAST_RESULTS = None       # BassKernelResults of most recent run (for test.py)


def build_bass():
    nc = bacc.Bacc("TRN2")

    gt = nc.dram_tensor("gt", [BL, 128, NCHUNK, GC], F32, kind="ExternalInput")
    f_in = nc.dram_tensor("f_in", [BL // 2, 128, HW], F32, kind="ExternalInput")
    amat = nc.dram_tensor("amat", [DIM, NJ, K], F32, kind="ExternalInput")
    wh = nc.dram_tensor("wh", [K, DQ, DIM], F32, kind="ExternalInput")
    id8 = nc.dram_tensor("id8", [DQ, DQ], F32, kind="ExternalInput")

    sel = nc.dram_tensor("sel", [BL, DQ, HW], F32, kind="ExternalOutput")
    codes = nc.dram_tensor("codes", [BL, 8], U32, kind="ExternalOutput")
    loss = nc.dram_tensor("loss", [BL, 1], F32, kind="ExternalOutput")

    with TileContext(nc) as tc:
        with tc.tile_pool(name="consts", bufs=1) as consts, \
             tc.tile_pool(name="gtp", bufs=8) as gtp, \
             tc.tile_pool(name="fpool", bufs=4) as fpool, \
             tc.tile_pool(name="stats", bufs=1) as statsp, \
             tc.tile_pool(name="small", bufs=2) as small, \
             tc.tile_pool(name="selout", bufs=2) as seloutp, \
             tc.tile_pool(name="ps_stats", bufs=2, space="PSUM") as ps_stats, \
             tc.tile_pool(name="ps_misc", bufs=1, space="PSUM") as ps_misc, \
             tc.tile_pool(name="ps_sel", bufs=3, space="PSUM") as ps_sel:

            a_sb = consts.tile([DIM, NJ, K], F32)
            nc.sync.dma_start(out=a_sb, in_=amat[:])
            wh_sb = consts.tile([K, DQ, DIM], F32)
            nc.sync.dma_start(out=wh_sb, in_=wh[:])
            id8_sb = consts.tile([DQ, DQ], F32)
            nc.sync.dma_start(out=id8_sb, in_=id8[:])

            # f rows resident for the whole kernel, two rows stacked per tile
            fpairs = []
            for p in range(BL // 2):
                fpair = fpool.tile([128, HW], F32)
                nc.sync.dma_start(out=fpair, in_=f_in[p])
                fpairs.append(fpair)

            # ---- phase 1: per-row stats  g g^T  (72x72) ----
            stats_sb = statsp.tile([GC, BL, GC], F32)
            for r in range(BL):
                gtt = gtp.tile([128, NCHUNK, GC], F32)
                nc.sync.dma_start(out=gtt, in_=gt[r])
                st_ps = ps_stats.tile([GC, GC], F32)
                for c in range(NCHUNK):
                    nc.tensor.matmul(
                        st_ps, lhsT=gtt[:, c, :], rhs=gtt[:, c, :],
                        start=(c == 0), stop=(c == NCHUNK - 1),
                    )
                nc.vector.tensor_copy(out=stats_sb[:, r, :], in_=st_ps)

            # qq block to partitions 0..7 (lane move => DMA)
            qstats = small.tile([DQ, BL, DQ], F32)
            nc.gpsimd.dma_start(out=qstats, in_=stats_sb[DIM:GC, :, DIM:GC])

            # ---- phase 2: distances [8 rows, 64 codes] ----
            dist_ps = ps_misc.tile([BL, K], F32)
            for j in range(NJ):
                if j < GC:
                    lhsT = stats_sb[0:DIM, :, j]      # [64, 8]
                    rhs = a_sb[:, j, :]               # [64, 64]
                else:
                    lhsT = qstats[:, :, j - GC]       # [8, 8]
                    rhs = a_sb[0:DQ, j, :]            # [8, 64]
                nc.tensor.matmul(dist_ps, lhsT=lhsT, rhs=rhs,
                                 start=(j == 0), stop=(j == NJ - 1))

            # ---- phase 3: argmin, codes, loss ----
            negd = small.tile([BL, K], F32)
            nc.vector.tensor_scalar_mul(negd, dist_ps, -1.0)
            maxv = small.tile([BL, 8], F32)
            maxi = small.tile([BL, 8], U32)
            nc.vector.max_with_indices(maxv, maxi, negd)
            nc.sync.dma_start(out=codes[:], in_=maxi)
            lossrow = small.tile([BL, 1], F32)
            nc.vector.tensor_scalar_mul(lossrow, maxv[:, 0:1], -1.0)
            nc.sync.dma_start(out=loss[:], in_=lossrow)

            mask = small.tile([BL, K], F32)
            nc.vector.tensor_scalar(
                mask, negd, maxv[:, 0:1], None, op0=mybir.AluOpType.is_ge,
            )

            # ---- phase 4: gather selected weights via one-hot matmuls ----
            maskt_ps = ps_misc.tile([K, BL], F32)
            nc.tensor.transpose(maskt_ps, mask, id8_sb)
            maskt = small.tile([K, BL], F32)
            nc.vector.tensor_copy(out=maskt, in_=maskt_ps)

            # selected weights, replicated to both partition halves so the
            # selection matmuls can pair with either half of an fpair tile
            wsel_ps = ps_misc.tile([128, DQ, BL], F32)
            for dq in range(DQ):
                nc.tensor.matmul(wsel_ps[0:DIM, dq, :], lhsT=wh_sb[:, dq, :],
                                 rhs=maskt, start=True, stop=True,
                                 tile_position=(0, 0))
                nc.tensor.matmul(wsel_ps[DIM:128, dq, :], lhsT=wh_sb[:, dq, :],
                                 rhs=maskt, start=True, stop=True,
                                 tile_position=(0, 64))
            wsel = small.tile([128, DQ, BL], F32)
            nc.vector.tensor_copy(out=wsel, in_=wsel_ps)

            # ---- phase 5: sel_b = Wsel_b @ f_b ----
            for r in range(BL):
                fpair = fpairs[r // 2]
                pbase = (r % 2) * DIM
                fhalf = fpair[pbase:pbase + DIM, :]
                sel_sb = seloutp.tile([DQ, NFC, 512], F32)
                for c in range(NFC):
                    sp = ps_sel.tile([DQ, 512], F32)
                    nc.tensor.matmul(sp, lhsT=wsel[pbase:pbase + DIM, :, r],
                                     rhs=fhalf[:, c * 512:(c + 1) * 512],
                                     start=True, stop=True,
                                     tile_position=(pbase, 0))
                    # alternate copy engine: DVE and ACT are both ~0.5us for a
                    # partition-starved [8,512] copy; splitting halves the wall
                    if c % 2 == 0:
                        nc.vector.tensor_copy(out=sel_sb[:, c, :], in_=sp)
                    else:
                        nc.scalar.copy(out=sel_sb[:, c, :], in_=sp)
                nc.sync.dma_start(out=sel[r], in_=sel_sb)

    return nc


def host_prep(features, query, weight):
    """Host-side layout prep + tiny weight-derived constants (all O(MB))."""
    f = np.ascontiguousarray(features, dtype=np.float32).reshape(B, DIM, HW)
    q = np.ascontiguousarray(query, dtype=np.float32).reshape(B, DQ, HW)
    w = np.ascontiguousarray(weight, dtype=np.float32)

    # gT[b] = [f_b; q_b]^T laid out [128 partitions, chunk, col]
    g = np.concatenate([f, q], axis=1)                    # [B, 72, HW]
    gt = np.ascontiguousarray(g.transpose(0, 2, 1))       # [B, HW, 72]
    gt = gt.reshape(B, NCHUNK, 128, GC).transpose(0, 2, 1, 3)  # [B,128,chunk,72]
    gt = np.ascontiguousarray(gt)

    w64 = w.astype(np.float64)
    m = np.einsum("kqc,kqd->kcd", w64, w64)               # [K, 64, 64]
    amat = np.zeros((DIM, NJ, K), dtype=np.float64)
    # j < 64: contraction row c' of  sum_{c,c'} M[k,c,c'] C[c',c]  (M symmetric)
    amat[:, :DIM, :] = m.transpose(2, 1, 0)               # amat[c',j=c,k]=M[k,c,c']
    # j = 64+dq: -2 W[k,dq,c]
    amat[:, DIM:GC, :] = -2.0 * w64.transpose(2, 1, 0)    # amat[c,64+dq,k]
    # j = 72+dq: qq diagonal pickup, ones row at partition dq
    for dq in range(DQ):
        amat[dq, GC + dq, :] = 1.0
    amat = np.ascontiguousarray(amat, dtype=np.float32)

    fin = np.ascontiguousarray(f.reshape(NCORES, BL // 2, 2 * DIM, HW))
    id8 = np.eye(DQ, dtype=np.float32)

    in_maps = []
    for c in range(NCORES):
        in_maps.append({
            "gt": np.ascontiguousarray(gt[c * BL:(c + 1) * BL]),
            "f_in": fin[c],
            "amat": amat,
            "wh": w,
            "id8": id8,
        })
    return in_maps


def kernel(features, query, weight):
    global LAST_RESULTS
    in_maps = host_prep(features, query, weight)
    nc = build_bass()
    nc.finalize()
    res = run_bass_kernel_spmd(
        nc, in_maps, core_ids=list(range(NCORES)), trace=TRACE,
    )
    LAST_RESULTS = res

    sel = np.empty((B, DQ, H, W), dtype=np.float32)
    codes = np.empty((B,), dtype=np.int32)
    loss_sum = 0.0
    for c in range(NCORES):
        r = res.results[c]
        sel[c * BL:(c + 1) * BL] = r["sel"].reshape(BL, DQ, H, W)
        codes[c * BL:(c + 1) * BL] = r["codes"][:, 0].astype(np.int32)
        loss_sum += float(r["loss"].sum())
    commit_loss = np.float32(loss_sum / (B * DQ * H * W))
    return sel, codes, commit_loss


# revision 14
# speedup vs baseline: 1.2073x; 1.2073x over previous
"""Trainium2 Bass kernel for nn_GuidedSampler (vq_codebook).

Math: reference computes, per batch row b
    kv[k]   = W_k @ f_b                (K=64 candidate 1x1-conv outputs)
    d2[b,k] = ||kv[k] - q_b||^2
    code_b  = argmin_k d2[b,k]
    sel_b   = kv[code_b]
    loss    = mean((sel - q)^2) = sum_b d2[b, code_b] / (B*DQ*H*W)

Instead of materializing kv (512 MB), use the Gram identity
    d2[b,k] = <M_k, C_b> - 2<W_k, G_b> + ||q_b||^2
with  M_k = W_k^T W_k   (host-precomputed, k-indexed 64x64)
      [C_b G_b; G_b^T qq_b] = g_b g_b^T,  g_b = [f_b; q_b]  (72 x HW)
computed on-device as 32 accumulating 128-contraction matmuls over a
host-pre-transposed gT layout.  Then 80 small accumulating matmuls with a
host-built constant A contract the stats against (M | -2W | I_qq) to give the
full distance matrix [8 rows, 64 codes] per core, argmin via max_with_indices,
and a final gather-free selection pass sel_b = (sum_k onehot W_k^T)^T f_b.

Sharding: data-parallel over batch, 8 rows per core, weights replicated.
"""

import numpy as np

import concourse.bass as bass
import concourse.bacc as bacc
from concourse import mybir
from concourse.tile import TileContext
from concourse.bass_utils import run_bass_kernel_spmd

B, DIM, DQ, H, W, K = 64, 64, 8, 64, 64, 64
HW = H * W                # 4096
NCORES = 8
BL = B // NCORES          # 8 batch rows per core
GC = DIM + DQ             # 72 stats dim
NCHUNK = HW // 128        # 32 contraction chunks for stats
NJ = DIM + DQ + DQ        # 80 distance-accumulation steps
NFC = HW // 512           # 8 free-dim chunks for the selection matmul

F32 = mybir.dt.float32
U32 = mybir.dt.uint32

TRACE = False             # set by test.py for profiling runs
LAST_RESULTS = None       # BassKernelResults of most recent run (for test.py)


def build_bass():
    nc = bacc.Bacc("TRN2")

    gt = nc.dram_tensor("gt", [BL, 128, NCHUNK, GC], F32, kind="ExternalInput")
    f_in = nc.dram_tensor("f_in", [BL // 2, 128, HW], F32, kind="ExternalInput")
    amat = nc.dram_tensor("amat", [DIM, NJ, K], F32, kind="ExternalInput")
    wh = nc.dram_tensor("wh", [K, DQ, DIM], F32, kind="ExternalInput")
    id8 = nc.dram_tensor("id8", [DQ, DQ], F32, kind="ExternalInput")

    sel = nc.dram_tensor("sel", [BL, DQ, HW], F32, kind="ExternalOutput")
    codes = nc.dram_tensor("codes", [BL, 8], U32, kind="ExternalOutput")
    loss = nc.dram_tensor("loss", [BL, 1], F32, kind="ExternalOutput")

    with TileContext(nc) as tc:
        with tc.tile_pool(name="consts", bufs=1) as consts, \
             tc.tile_pool(name="gtp", bufs=8) as gtp, \
             tc.tile_pool(name="fpool", bufs=4) as fpool, \
             tc.tile_pool(name="stats", bufs=1) as statsp, \
             tc.tile_pool(name="small", bufs=2) as small, \
             tc.tile_pool(name="selout", bufs=2) as seloutp, \
             tc.tile_pool(name="ps_stats", bufs=2, space="PSUM") as ps_stats, \
             tc.tile_pool(name="ps_misc", bufs=1, space="PSUM") as ps_misc, \
             tc.tile_pool(name="ps_sel", bufs=3, space="PSUM") as ps_sel:

            wh_sb = consts.tile([K, DQ, DIM], F32)
            nc.sync.dma_start(out=wh_sb, in_=wh[:])
            id8_sb = consts.tile([DQ, DQ], F32)
            nc.sync.dma_start(out=id8_sb, in_=id8[:])

            # ---- phase 1: per-row stats  g g^T  (72x72) ----
            stats_sb = statsp.tile([GC, BL, GC], F32)
            for r in range(BL):
                gtt = gtp.tile([128, NCHUNK, GC], F32)
                nc.sync.dma_start(out=gtt, in_=gt[r])
                st_ps = ps_stats.tile([GC, GC], F32)
                for c in range(NCHUNK):
                    nc.tensor.matmul(
                        st_ps, lhsT=gtt[:, c, :], rhs=gtt[:, c, :],
                        start=(c == 0), stop=(c == NCHUNK - 1),
                    )
                nc.vector.tensor_copy(out=stats_sb[:, r, :], in_=st_ps)

            # a_sb (dist phase) and f (sel phase) are consumed later — issue
            # their DMAs after the gt streams so stats is never input-starved
            a_sb = consts.tile([DIM, NJ, K], F32)
            nc.sync.dma_start(out=a_sb, in_=amat[:])
            fpairs = []
            for p in range(BL // 2):
                fpair = fpool.tile([128, HW], F32)
                nc.sync.dma_start(out=fpair, in_=f_in[p])
                fpairs.append(fpair)

            # qq block to partitions 0..7 (lane move => DMA)
            qstats = small.tile([DQ, BL, DQ], F32)
            nc.gpsimd.dma_start(out=qstats, in_=stats_sb[DIM:GC, :, DIM:GC])

            # ---- phase 2: distances [8 rows, 64 codes] ----
            dist_ps = ps_misc.tile([BL, K], F32)
            for j in range(NJ):
                if j < GC:
                    lhsT = stats_sb[0:DIM, :, j]      # [64, 8]
                    rhs = a_sb[:, j, :]               # [64, 64]
                else:
                    lhsT = qstats[:, :, j - GC]       # [8, 8]
                    rhs = a_sb[0:DQ, j, :]            # [8, 64]
                nc.tensor.matmul(dist_ps, lhsT=lhsT, rhs=rhs,
                                 start=(j == 0), stop=(j == NJ - 1))

            # ---- phase 3: argmin, codes, loss ----
            negd = small.tile([BL, K], F32)
            nc.vector.tensor_scalar_mul(negd, dist_ps, -1.0)
            maxv = small.tile([BL, 8], F32)
            maxi = small.tile([BL, 8], U32)
            nc.vector.max_with_indices(maxv, maxi, negd)
            nc.sync.dma_start(out=codes[:], in_=maxi)
            lossrow = small.tile([BL, 1], F32)
            nc.vector.tensor_scalar_mul(lossrow, maxv[:, 0:1], -1.0)
            nc.sync.dma_start(out=loss[:], in_=lossrow)

            mask = small.tile([BL, K], F32)
            nc.vector.tensor_scalar(
                mask, negd, maxv[:, 0:1], None, op0=mybir.AluOpType.is_ge,
            )

            # ---- phase 4: gather selected weights via one-hot matmuls ----
            maskt_ps = ps_misc.tile([K, BL], F32)
            nc.tensor.transpose(maskt_ps, mask, id8_sb)
            maskt = small.tile([K, BL], F32)
            nc.vector.tensor_copy(out=maskt, in_=maskt_ps)

            # selected weights, replicated to both partition halves so the
            # selection matmuls can pair with either half of an fpair tile
            wsel_ps = ps_misc.tile([128, DQ, BL], F32)
            for dq in range(DQ):
                nc.tensor.matmul(wsel_ps[0:DIM, dq, :], lhsT=wh_sb[:, dq, :],
                                 rhs=maskt, start=True, stop=True,
                                 tile_position=(0, 0))
                nc.tensor.matmul(wsel_ps[DIM:128, dq, :], lhsT=wh_sb[:, dq, :],
                                 rhs=maskt, start=True, stop=True,
                                 tile_position=(0, 64))
            wsel = small.tile([128, DQ, BL], F32)
            nc.vector.tensor_copy(out=wsel, in_=wsel_ps)

            # ---- phase 5: sel_b = Wsel_b @ f_b ----
            for r in range(BL):
                fpair = fpairs[r // 2]
                pbase = (r % 2) * DIM
                fhalf = fpair[pbase:pbase + DIM, :]
                sel_sb = seloutp.tile([DQ, NFC, 512], F32)
                for c in range(NFC):
                    sp = ps_sel.tile([DQ, 512], F32)
                    nc.tensor.matmul(sp, lhsT=wsel[pbase:pbase + DIM, :, r],
                                     rhs=fhalf[:, c * 512:(c + 1) * 512],
                                     start=True, stop=True,
                                     tile_position=(pbase, 0))
                    # alternate copy engine: DVE and ACT are both ~0.5us for a
                    # partition-starved [8,512] copy; splitting halves the wall
                    if c % 2 == 0:
                        nc.vector.tensor_copy(out=sel_sb[:, c, :], in_=sp)
                    else:
                        nc.scalar.copy(out=sel_sb[:, c, :], in_=sp)
                nc.sync.dma_start(out=sel[r], in_=sel_sb)

    return nc


def host_prep(features, query, weight):
    """Host-side layout prep + tiny weight-derived constants (all O(MB))."""
    f = np.ascontiguousarray(features, dtype=np.float32).reshape(B, DIM, HW)
    q = np.ascontiguousarray(query, dtype=np.float32).reshape(B, DQ, HW)
    w = np.ascontiguousarray(weight, dtype=np.float32)

    # gT[b] = [f_b; q_b]^T laid out [128 partitions, chunk, col]
    g = np.concatenate([f, q], axis=1)                    # [B, 72, HW]
    gt = np.ascontiguousarray(g.transpose(0, 2, 1))       # [B, HW, 72]
    gt = gt.reshape(B, NCHUNK, 128, GC).transpose(0, 2, 1, 3)  # [B,128,chunk,72]
    gt = np.ascontiguousarray(gt)

    w64 = w.astype(np.float64)
    m = np.einsum("kqc,kqd->kcd", w64, w64)               # [K, 64, 64]
    amat = np.zeros((DIM, NJ, K), dtype=np.float64)
    # j < 64: contraction row c' of  sum_{c,c'} M[k,c,c'] C[c',c]  (M symmetric)
    amat[:, :DIM, :] = m.transpose(2, 1, 0)               # amat[c',j=c,k]=M[k,c,c']
    # j = 64+dq: -2 W[k,dq,c]
    amat[:, DIM:GC, :] = -2.0 * w64.transpose(2, 1, 0)    # amat[c,64+dq,k]
    # j = 72+dq: qq diagonal pickup, ones row at partition dq
    for dq in range(DQ):
        amat[dq, GC + dq, :] = 1.0
    amat = np.ascontiguousarray(amat, dtype=np.float32)

    fin = np.ascontiguousarray(f.reshape(NCORES, BL // 2, 2 * DIM, HW))
    id8 = np.eye(DQ, dtype=np.float32)

    in_maps = []
    for c in range(NCORES):
        in_maps.append({
            "gt": np.ascontiguousarray(gt[c * BL:(c + 1) * BL]),
            "f_in": fin[c],
            "amat": amat,
            "wh": w,
            "id8": id8,
        })
    return in_maps


def kernel(features, query, weight):
    global LAST_RESULTS
    in_maps = host_prep(features, query, weight)
    nc = build_bass()
    nc.finalize()
    res = run_bass_kernel_spmd(
        nc, in_maps, core_ids=list(range(NCORES)), trace=TRACE,
    )
    LAST_RESULTS = res

    sel = np.empty((B, DQ, H, W), dtype=np.float32)
    codes = np.empty((B,), dtype=np.int32)
    loss_sum = 0.0
    for c in range(NCORES):
        r = res.results[c]
        sel[c * BL:(c + 1) * BL] = r["sel"].reshape(BL, DQ, H, W)
        codes[c * BL:(c + 1) * BL] = r["codes"][:, 0].astype(np.int32)
        loss_sum += float(r["loss"].sum())
    commit_loss = np.float32(loss_sum / (B * DQ * H * W))
    return sel, codes, commit_loss
